# revision 18
# baseline (speedup 1.0000x reference)
"""Trainium2 Bass kernel for the fused L2-embed / RMS-norm / tanh-gate module.

  sumsq[n,c] = sum_{h,w} x[n,c,h,w]^2
  embed      = sqrt(sumsq + eps) * alpha
  inv[n]     = rsqrt(mean_c(embed^2) + eps)
  z          = embed * gamma * inv + beta
  out        = x * (1 + tanh(z))

Data-parallel over the batch axis: 8 samples per NeuronCore, 8 cores.
The problem is HBM-bound (read x, write out, trivial math), and the 2e-2
rel-err budget comfortably admits bf16 transport (~2e-3 norm error), so x
ships to the device as bf16 and the output returns as bf16 — halving HBM
traffic vs fp32.  Per half-sample (0.8 MB) the kernel streams x in,
square-accumulates on ScalarE (fp32 accum), does the tiny per-sample
stage-B chain on VectorE/PE in fp32 (rsqrt via Newton iteration to avoid
ACT table switches; tanh is the only table-loaded ACT function), applies
the gate with a tensor_scalar multiply in-place, and streams the result
out.  Loads ride the sync/scalar DMA rings, stores ride vector/tensor, so
stores never head-of-line-block later loads; all 8 sample buffers fit in
SBUF so every load can be queued immediately.
"""

import json

import numpy as np

N, C, H, W = 64, 256, 56, 56
HW = H * W                    # 3136
NCORES = 8
NPC = N // NCORES             # samples per core
EPS = 1e-5
P = 128
K = C // P                    # free-dim channel halves per partition (2)
RSQRT_MAGIC = 0x5F3759DF

_cache = {}


# --------------------------------------------------------------------------
# BIR post-processing: the walrus build in this container allows at most one
# sync wait and one sync update per instruction.  Hoist excess waits onto
# NoOps inserted before the instruction (same engine/block); move excess
# updates of non-DMA instructions onto a NoOp right after.
# --------------------------------------------------------------------------
_nop_counter = [0]


def _mk_nop(engine, waits, updates, debug=0):
    _nop_counter[0] += 1
    return {
        "name": f"I-wsplit-{_nop_counter[0]}",
        "opcode": "NoOp",
        "engine": engine,
        "ins": [],
        "outs": [],
        "debug": debug,
        "sync_info": {"on_wait": waits, "on_update": updates},
    }


def _split_sync_waits(bir_json_bytes):
    d = json.loads(bir_json_bytes)
    for f in d.get("functions", []):
        for blk in f.get("blocks", []):
            new_insts = []
            for inst in blk.get("instructions", []):
                si = inst.get("sync_info")
                after = []
                if si:
                    waits = list(si.get("on_wait") or [])
                    updates = list(si.get("on_update") or [])
                    eng = inst.get("engine")
                    dbg = inst.get("debug", 0)
                    if len(waits) > 1:
                        for w in waits[:-1]:
                            new_insts.append(_mk_nop(eng, [w], [], dbg))
                        waits = waits[-1:]
                    if len(updates) > 1:
                        op = inst.get("opcode", "")
                        if "DMA" in op:
                            raise RuntimeError(
                                f"DMA instruction {inst.get('name')} has "
                                f"{len(updates)} sync updates; cannot split"
                            )
                        for u in updates[1:]:
                            after.append(_mk_nop(eng, [], [u], dbg))
                        updates = updates[:1]
                    si["on_wait"] = waits
                    si["on_update"] = updates
                new_insts.append(inst)
                new_insts.extend(after)
            blk["instructions"] = new_insts
    return json.dumps(d).encode()


def _patch_bass(nc):
    orig = nc.to_json_bytes

    def fixed(*a, **kw):
        return _split_sync_waits(orig(*a, **kw))

    nc.to_json_bytes = fixed
    return nc


# --------------------------------------------------------------------------
# Kernel build
# --------------------------------------------------------------------------
def _build():
    import concourse.bass as bass
    import concourse.tile as tile
    from concourse import mybir
    from concourse.tile import ScopedClock

    f32 = mybir.dt.float32
    bf16 = mybir.dt.bfloat16
    u32 = mybir.dt.uint32
    Alu = mybir.AluOpType
    Act = mybir.ActivationFunctionType

    class LeanExitTileContext(tile.TileContext):
        """Standard exit minus the second all-engine barrier (~3.4us).
        NRT only starts a subsequent execution after every engine stream has
        ended, and the sem clears sit on gpsimd's own stream, so the final
        barrier adds no ordering we need."""

        def _drain_and_barrier(self, tick_clock, wait_clock):
            drain_inst = self.nc.sync.drain()
            wait_clock.add_sem_waits(
                drain_inst.ins, ScopedClock({None: tick_clock.global_clock})
            )
            self.nc.all_engine_barrier()
            assert self.sems is not None
            popped = self.nc._tile_sem_poison_stack.pop()
            assert popped is self._sem_poison
            self.nc.clear_and_free_semaphores(
                list(self.sems.allocated().values())
            )

    nc = bass.Bass(trn_type="TRN2")
    x = nc.dram_tensor("x", [NPC, C, HW], bf16, kind="ExternalInput")
    alpha = nc.dram_tensor("alpha", [C], f32, kind="ExternalInput")
    gamma = nc.dram_tensor("gamma", [C], f32, kind="ExternalInput")
    beta = nc.dram_tensor("beta", [C], f32, kind="ExternalInput")
    out = nc.dram_tensor("out", [NPC, C, HW], bf16, kind="ExternalOutput")

    with LeanExitTileContext(nc) as tc:
        with (
            tc.tile_pool(name="xpool", bufs=1) as xpool,
            tc.tile_pool(name="scratch", bufs=1) as scratch,
            tc.tile_pool(name="small", bufs=6) as small,
            tc.tile_pool(name="singles", bufs=1) as singles,
            tc.tile_pool(name="ps", bufs=4, space="PSUM") as ps,
        ):
            # ---- one-time constants ----
            # channel c lives at (partition c//K, free-half c%K).
            # Params ride gpsimd SWDGE (its own queue row): each [P, K] param
            # is 128 tiny descriptors, which on a HWDGE ring would delay the
            # first x-loads' descriptor generation. Not needed until sample
            # 0's stage B (~10us in), so gpsimd's slow preamble is harmless.
            a_col = singles.tile([P, K], f32)
            nc.gpsimd.dma_start(out=a_col[:], in_=alpha[:].rearrange("(p a) -> p a", p=P))
            g_col = singles.tile([P, K], f32)
            nc.gpsimd.dma_start(out=g_col[:], in_=gamma[:].rearrange("(p a) -> p a", p=P))
            b_col = singles.tile([P, K], f32)
            nc.gpsimd.dma_start(out=b_col[:], in_=beta[:].rearrange("(p a) -> p a", p=P))
            zero_bias = singles.tile([P, 1], f32)  # memset, not const-DMA:
            nc.vector.memset(zero_bias[:], 0.0)    # keeps ACT off the const
            # tensor DMA dependency that otherwise delays the first square

            # Dummy 1-element activation: pulls the ~1.4us ACT function
            # table load into the DMA preamble instead of paying for it
            # between the load triggers and the first real square.
            act_warm = singles.tile([P, 1], f32)
            nc.scalar.activation(
                out=act_warm[:], in_=zero_bias[:], func=Act.Square,
                bias=zero_bias[:, 0:1],
            )

            a2_col = singles.tile([P, K], f32)       # alpha^2
            nc.vector.tensor_mul(a2_col[:], a_col[:], a_col[:])
            ag_col = singles.tile([P, K], f32)       # alpha*gamma
            nc.vector.tensor_mul(ag_col[:], a_col[:], g_col[:])

            ones_t = singles.tile([P, P], f32)       # all-ones lhsT for col-sum
            nc.vector.memset(ones_t[:], 1.0)
            magic = singles.tile([P, K], u32)        # rsqrt seed constant
            nc.vector.memset(magic[:], RSQRT_MAGIC)

            # ---- DMA plan.  Loads: whole-sample transfers (one 12.5KB
            # contiguous descriptor per partition) alternating between the
            # two HWDGE rings (sync=0 / scalar=1 — the only DMA engines
            # besides gpsimd SWDGE); the first two samples are split into
            # half-sample transfers, one per ring, so the first square
            # starts ~4us earlier.  Ring queue depth is 4, so exactly four
            # items per ring are triggered up front and L6/L7 fire from
            # iterations 0/1 once slots free — a trigger on a full ring
            # stalls the issuing sequencer, and the scalar sequencer also
            # runs the squares.
            #
            # Stores: samples 0-3 ride gpsimd's SWDGE queue (idle sequencer,
            # and the 16 DMA channels execute descriptors from any queue);
            # samples 4-7 ride the HW rings, whose sequencers are idle by
            # then — SWDGE completion semaphores resolve ~7us after the data
            # lands, and that latency must not sit on the exit drain for the
            # final stores.
            ring = (nc.sync, nc.scalar)
            xts, outs = [], []
            for n in range(NPC):
                xts.append(xpool.tile([P, K, HW], bf16, name=f"xt{n}"))
                outs.append(out[n].rearrange("(p a) hw -> p a hw", p=P))

            def load(n):
                xr = x[n].rearrange("(p a) hw -> p a hw", p=P)
                if n < 2:
                    for k in range(K):
                        ring[(n + k) % 2].dma_start(out=xts[n][:, k], in_=xr[:, k])
                else:
                    ring[n % 2].dma_start(out=xts[n][:], in_=xr[:])

            def store(n):
                # half-tile stores: two 6.3KB descriptors per partition move
                # ~8% faster through a channel than one 12.5KB descriptor
                for k in range(K):
                    if n >= 4:
                        ring[(n + k) % 2].dma_start(
                            out=outs[n][:, k], in_=xts[n][:, k]
                        )
                    else:
                        nc.gpsimd.dma_start(out=outs[n][:, k], in_=xts[n][:, k])

            for n in range(NPC - 2):
                load(n)

            # tanh via odd deg-7 polynomial: |z| <= ~0.5 for this module
            # (gamma ~ 0.1*randn, rms-normalized embed), where the fit errs
            # < 1e-4 — invisible next to the 2.4e-3 bf16 transport error.
            # No ACT-table tanh means the gate chain never touches the
            # square-backlogged ScalarE.
            TC3, TC2, TC1, TC0 = (
                -0.008266237707336609,
                0.0772699109198538,
                -0.3081753480963571,
                0.9992571512344053,
            )

            sq_act = scratch.tile([P, K, HW], bf16)   # ACT square dummy out
            sq_dve = scratch.tile([P, HW], bf16)      # DVE fused-square dummy

            def chain(u_t, E):
                """sumsq -> gate, entirely on engine E (Pool for samples
                0-5, DVE for the tail) except the PE column-sum and the DVE
                reciprocal (Pool has no reciprocal).  ~25 two-element ops:
                ~4.8us on Pool, ~2.1us on DVE — Pool absorbs the steady-
                state chains so the DVE can keep up with its 16 gate
                multiplies, the only bulk op with a fast (2x) mode."""
                ua = small.tile([P, K], f32, name="ua")
                E.tensor_scalar(ua[:], u_t[:], EPS, None, op0=Alu.add)
                E.tensor_mul(ua[:], ua[:], a2_col[:])
                cs = ps.tile([P, K], f32, name="cs")
                nc.tensor.matmul(cs[:], ones_t[:], ua[:], start=True, stop=True)
                msum = small.tile([P, 1], f32, name="msum")
                # PSUM is DVE/ACT-visible only — Pool can't read the matmul
                # result, so this one add stays on the DVE (as does the
                # reciprocal below, which Pool lacks entirely)
                nc.vector.tensor_reduce(
                    msum[:], cs[:], axis=mybir.AxisListType.X, op=Alu.add
                )
                v_t = small.tile([P, 1], f32, name="v_t")
                E.tensor_scalar(
                    v_t[:], msum[:], 1.0 / C, EPS, op0=Alu.mult, op1=Alu.add
                )
                rv = small.tile([P, 1], f32, name="rv")
                nc.vector.reciprocal(rv[:], v_t[:])
                w_t = small.tile([P, K], f32, name="w_t")
                E.tensor_scalar(
                    w_t[:], u_t[:], EPS, rv[:, 0:1], op0=Alu.add, op1=Alu.mult
                )
                # y ~= rsqrt(w): bit-trick seed + 1 Newton step (seed err
                # ~3% -> ~0.2%; |dz| stays ~1e-3, far inside budget)
                y_t = small.tile([P, K], f32, name="y_t")
                sh = small.tile([P, K], u32, name="sh")
                # integer shift/subtract are DVE-only ops on TRN2
                nc.vector.tensor_scalar(
                    sh[:], w_t[:].bitcast(u32), 1, None,
                    op0=Alu.logical_shift_right,
                )
                nc.vector.tensor_tensor(
                    out=y_t[:].bitcast(u32), in0=magic[:], in1=sh[:],
                    op=Alu.subtract,
                )
                t_t = small.tile([P, K], f32, name="t_t")
                E.tensor_mul(t_t[:], w_t[:], y_t[:])
                E.tensor_mul(t_t[:], t_t[:], y_t[:])
                E.tensor_scalar(t_t[:], t_t[:], -0.5, 1.5, op0=Alu.mult, op1=Alu.add)
                E.tensor_mul(y_t[:], y_t[:], t_t[:])
                # z = alpha*gamma*sqrt(w) + beta ; sqrt(w) = w * rsqrt(w)
                z_t = small.tile([P, K], f32, name="z_t")
                E.tensor_mul(z_t[:], w_t[:], y_t[:])
                E.tensor_mul(z_t[:], z_t[:], ag_col[:])
                E.tensor_add(z_t[:], z_t[:], b_col[:])
                return z_t

            z_ts = {}

            def finish(m):
                # gate = 1 + z*P(z^2), then apply in place and store —
                # emitted one iteration behind the z-chain so the scheduler
                # sees sample m's gate work before sample m+1's z-chain
                E = nc.vector
                z_t = z_ts.pop(m)
                t2 = small.tile([P, K], f32, name="t2")
                E.tensor_mul(t2[:], z_t[:], z_t[:])
                h = small.tile([P, K], f32, name="h")
                E.tensor_scalar(h[:], t2[:], TC3, TC2, op0=Alu.mult, op1=Alu.add)
                for c in (TC1, TC0):
                    E.tensor_mul(h[:], h[:], t2[:])
                    E.tensor_scalar(h[:], h[:], c, None, op0=Alu.add)
                gt = small.tile([P, K], f32, name="gt")
                E.tensor_mul(gt[:], h[:], z_t[:])
                E.tensor_scalar(gt[:], gt[:], 1.0, None, op0=Alu.add)
                for k in range(K):
                    nc.vector.tensor_scalar_mul(
                        xts[m][:, k], in0=xts[m][:, k], scalar1=gt[:, k : k + 1]
                    )
                store(m)

            for n in range(NPC):
                xt = xts[n]
                u_t = small.tile([P, K], f32)   # raw per-channel sumsq
                # ---- stage A: measured rates per half: ACT square+accum
                # 2.8us, DVE fused mult+row-sum 3.3us, Pool ~11us (unusable).
                # ACT takes 13 halves (samples 0-4 whole + k0 of 5-7), DVE
                # takes k1 of 5-7 — the tail samples' halves run on two
                # engines in parallel because everything after the last load
                # is critical path. ----
                split = n >= 5
                for k in range(1) if split else range(K):
                    nc.scalar.activation(
                        out=sq_act[:, k],
                        in_=xt[:, k],
                        func=Act.Square,
                        bias=zero_bias[:, 0:1],
                        accum_out=u_t[:, k : k + 1],
                    )
                if n < 2:
                    load(n + 6)
                if split:
                    nc.vector.scalar_tensor_tensor(
                        out=sq_dve[:],
                        in0=xt[:, 1],
                        scalar=1.0,
                        in1=xt[:, 1],
                        op0=Alu.mult,
                        op1=Alu.mult,
                        accum_out=u_t[:, 1:2],
                    )
                z_ts[n] = chain(u_t, nc.vector)
                if n >= 1:
                    finish(n - 1)

            finish(NPC - 1)

    return _patch_bass(nc)


def _get_nc():
    if "nc" not in _cache:
        _cache["nc"] = _build()
    return _cache["nc"]


def _ensure_axon_hooks_stub():
    """bass_utils imports antenv.axon_hooks when tracing is requested (e.g.
    via a stray BASS_TRACE=1); this image lacks that module. Provide a stub
    whose hook getter returns None so the untraced fallback path runs."""
    import sys
    import types

    try:
        import antenv.axon_hooks  # noqa: F401
    except ImportError:
        mod = types.ModuleType("antenv.axon_hooks")
        _holder = [None]
        mod.set_axon_ntff_profile_hook = lambda h: _holder.__setitem__(0, h)
        mod.get_axon_ntff_profile_hook = lambda: _holder[0]
        sys.modules["antenv.axon_hooks"] = mod


def _run(x, alpha, gamma, beta, trace=False, **spmd_kwargs):
    import ml_dtypes

    from concourse.bass_utils import run_bass_kernel_spmd

    _ensure_axon_hooks_stub()

    nc = _get_nc()
    bf16 = ml_dtypes.bfloat16
    x = np.asarray(x).reshape(N, C, HW).astype(bf16)
    alpha = np.ascontiguousarray(np.asarray(alpha), dtype=np.float32)
    gamma = np.ascontiguousarray(np.asarray(gamma), dtype=np.float32)
    beta = np.ascontiguousarray(np.asarray(beta), dtype=np.float32)
    in_maps = [
        {
            "x": np.ascontiguousarray(x[c * NPC : (c + 1) * NPC]),
            "alpha": alpha,
            "gamma": gamma,
            "beta": beta,
        }
        for c in range(NCORES)
    ]
    res = run_bass_kernel_spmd(
        nc, in_maps, core_ids=list(range(NCORES)), trace=trace, **spmd_kwargs
    )
    full = np.concatenate([r["out"] for r in res.results], axis=0)
    return full.reshape(N, C, H, W).astype(np.float32), res


def kernel(x, alpha, gamma, beta):
    out, _ = _run(x, alpha, gamma, beta)
    return out


# revision 19
# speedup vs baseline: 1.2605x; 1.2605x over previous
"""Trainium2 Bass kernel for the fused L2-embed / RMS-norm / tanh-gate module.

  sumsq[n,c] = sum_{h,w} x[n,c,h,w]^2
  embed      = sqrt(sumsq + eps) * alpha
  inv[n]     = rsqrt(mean_c(embed^2) + eps)
  z          = embed * gamma * inv + beta
  out        = x * (1 + tanh(z))

Data-parallel over the batch axis: 8 samples per NeuronCore, 8 cores.
The problem is HBM-bound (read x, write out, trivial math), and the 2e-2
rel-err budget comfortably admits bf16 transport (~2e-3 norm error), so x
ships to the device as bf16 and the output returns as bf16 — halving HBM
traffic vs fp32.  Per half-sample (0.8 MB) the kernel streams x in,
square-accumulates on ScalarE (fp32 accum), does the tiny per-sample
stage-B chain on VectorE/PE in fp32 (rsqrt via Newton iteration to avoid
ACT table switches; tanh is the only table-loaded ACT function), applies
the gate with a tensor_scalar multiply in-place, and streams the result
out.  Loads ride the sync/scalar DMA rings, stores ride vector/tensor, so
stores never head-of-line-block later loads; all 8 sample buffers fit in
SBUF so every load can be queued immediately.
"""

import json

import numpy as np

N, C, H, W = 64, 256, 56, 56
HW = H * W                    # 3136
NCORES = 8
NPC = N // NCORES             # samples per core
EPS = 1e-5
P = 128
K = C // P                    # free-dim channel halves per partition (2)
RSQRT_MAGIC = 0x5F3759DF

_cache = {}


# --------------------------------------------------------------------------
# BIR post-processing: the walrus build in this container allows at most one
# sync wait and one sync update per instruction.  Hoist excess waits onto
# NoOps inserted before the instruction (same engine/block); move excess
# updates of non-DMA instructions onto a NoOp right after.
# --------------------------------------------------------------------------
_nop_counter = [0]


def _mk_nop(engine, waits, updates, debug=0):
    _nop_counter[0] += 1
    return {
        "name": f"I-wsplit-{_nop_counter[0]}",
        "opcode": "NoOp",
        "engine": engine,
        "ins": [],
        "outs": [],
        "debug": debug,
        "sync_info": {"on_wait": waits, "on_update": updates},
    }


def _split_sync_waits(bir_json_bytes):
    d = json.loads(bir_json_bytes)
    for f in d.get("functions", []):
        for blk in f.get("blocks", []):
            new_insts = []
            for inst in blk.get("instructions", []):
                si = inst.get("sync_info")
                after = []
                if si:
                    waits = list(si.get("on_wait") or [])
                    updates = list(si.get("on_update") or [])
                    eng = inst.get("engine")
                    dbg = inst.get("debug", 0)
                    if len(waits) > 1:
                        for w in waits[:-1]:
                            new_insts.append(_mk_nop(eng, [w], [], dbg))
                        waits = waits[-1:]
                    if len(updates) > 1:
                        op = inst.get("opcode", "")
                        if "DMA" in op:
                            raise RuntimeError(
                                f"DMA instruction {inst.get('name')} has "
                                f"{len(updates)} sync updates; cannot split"
                            )
                        for u in updates[1:]:
                            after.append(_mk_nop(eng, [], [u], dbg))
                        updates = updates[:1]
                    si["on_wait"] = waits
                    si["on_update"] = updates
                new_insts.append(inst)
                new_insts.extend(after)
            blk["instructions"] = new_insts
    return json.dumps(d).encode()


def _patch_bass(nc):
    orig = nc.to_json_bytes

    def fixed(*a, **kw):
        return _split_sync_waits(orig(*a, **kw))

    nc.to_json_bytes = fixed
    return nc


# --------------------------------------------------------------------------
# Kernel build
# --------------------------------------------------------------------------
def _build():
    import concourse.bass as bass
    import concourse.tile as tile
    from concourse import mybir
    from concourse.tile import ScopedClock

    f32 = mybir.dt.float32
    bf16 = mybir.dt.bfloat16
    u32 = mybir.dt.uint32
    Alu = mybir.AluOpType
    Act = mybir.ActivationFunctionType

    class LeanExitTileContext(tile.TileContext):
        """Standard exit minus the second all-engine barrier (~3.4us).
        NRT only starts a subsequent execution after every engine stream has
        ended, and the sem clears sit on gpsimd's own stream, so the final
        barrier adds no ordering we need."""

        def _drain_and_barrier(self, tick_clock, wait_clock):
            drain_inst = self.nc.sync.drain()
            wait_clock.add_sem_waits(
                drain_inst.ins, ScopedClock({None: tick_clock.global_clock})
            )
            self.nc.all_engine_barrier()
            assert self.sems is not None
            popped = self.nc._tile_sem_poison_stack.pop()
            assert popped is self._sem_poison
            self.nc.clear_and_free_semaphores(
                list(self.sems.allocated().values())
            )

    nc = bass.Bass(trn_type="TRN2")
    x = nc.dram_tensor("x", [NPC, C, HW], bf16, kind="ExternalInput")
    alpha = nc.dram_tensor("alpha", [C], f32, kind="ExternalInput")
    gamma = nc.dram_tensor("gamma", [C], f32, kind="ExternalInput")
    beta = nc.dram_tensor("beta", [C], f32, kind="ExternalInput")
    out = nc.dram_tensor("out", [NPC, C, HW], bf16, kind="ExternalOutput")

    with LeanExitTileContext(nc) as tc:
        with (
            tc.tile_pool(name="xpool", bufs=1) as xpool,
            tc.tile_pool(name="scratch", bufs=1) as scratch,
            tc.tile_pool(name="small", bufs=6) as small,
            tc.tile_pool(name="singles", bufs=1) as singles,
            tc.tile_pool(name="ps", bufs=4, space="PSUM") as ps,
        ):
            # ---- one-time constants ----
            # channel c lives at (partition c//K, free-half c%K).
            # Params ride gpsimd SWDGE (its own queue row): each [P, K] param
            # is 128 tiny descriptors, which on a HWDGE ring would delay the
            # first x-loads' descriptor generation. Not needed until sample
            # 0's stage B (~10us in), so gpsimd's slow preamble is harmless.
            a_col = singles.tile([P, K], f32)
            nc.gpsimd.dma_start(out=a_col[:], in_=alpha[:].rearrange("(p a) -> p a", p=P))
            g_col = singles.tile([P, K], f32)
            nc.gpsimd.dma_start(out=g_col[:], in_=gamma[:].rearrange("(p a) -> p a", p=P))
            b_col = singles.tile([P, K], f32)
            nc.gpsimd.dma_start(out=b_col[:], in_=beta[:].rearrange("(p a) -> p a", p=P))
            zero_bias = singles.tile([P, 1], f32)  # memset, not const-DMA:
            nc.vector.memset(zero_bias[:], 0.0)    # keeps ACT off the const
            # tensor DMA dependency that otherwise delays the first square

            # Dummy 1-element activation: pulls the ~1.4us ACT function
            # table load into the DMA preamble instead of paying for it
            # between the load triggers and the first real square.
            act_warm = singles.tile([P, 1], f32)
            nc.scalar.activation(
                out=act_warm[:], in_=zero_bias[:], func=Act.Square,
                bias=zero_bias[:, 0:1],
            )

            a2_col = singles.tile([P, K], f32)       # alpha^2
            nc.vector.tensor_mul(a2_col[:], a_col[:], a_col[:])
            ag_col = singles.tile([P, K], f32)       # alpha*gamma
            nc.vector.tensor_mul(ag_col[:], a_col[:], g_col[:])

            ones_t = singles.tile([P, P], f32)       # all-ones lhsT for col-sum
            nc.vector.memset(ones_t[:], 1.0)
            magic = singles.tile([P, K], u32)        # rsqrt seed constant
            nc.vector.memset(magic[:], RSQRT_MAGIC)

            # ---- DMA plan.  Loads: whole-sample transfers (one 12.5KB
            # contiguous descriptor per partition) alternating between the
            # two HWDGE rings (sync=0 / scalar=1 — the only DMA engines
            # besides gpsimd SWDGE); the first two samples are split into
            # half-sample transfers, one per ring, so the first square
            # starts ~4us earlier.  Ring queue depth is 4, so exactly four
            # items per ring are triggered up front and L6/L7 fire from
            # iterations 0/1 once slots free — a trigger on a full ring
            # stalls the issuing sequencer, and the scalar sequencer also
            # runs the squares.  Stores ride gpsimd's SWDGE queue (idle
            # sequencer; the 16 DMA channels execute descriptors from any
            # queue) as half-tile transfers, issued as soon as each sample's
            # gate multiply lands.
            ring = (nc.sync, nc.scalar)
            xts, outs = [], []
            for n in range(NPC):
                xts.append(xpool.tile([P, K, HW], bf16, name=f"xt{n}"))
                outs.append(out[n].rearrange("(p a) hw -> p a hw", p=P))

            def load(n):
                xr = x[n].rearrange("(p a) hw -> p a hw", p=P)
                if n < 2:
                    for k in range(K):
                        ring[(n + k) % 2].dma_start(out=xts[n][:, k], in_=xr[:, k])
                else:
                    ring[n % 2].dma_start(out=xts[n][:], in_=xr[:])

            def store(n):
                for k in range(K):
                    nc.gpsimd.dma_start(out=outs[n][:, k], in_=xts[n][:, k])

            for n in range(NPC - 2):
                load(n)

            z_ts = {}

            sq_act = scratch.tile([P, K, HW], bf16)   # ACT square dummy out
            sq_dve = scratch.tile([P, HW], bf16)      # DVE fused-square dummy

            def finish(m):
                z_t = z_ts.pop(m)
                gt = small.tile([P, K], f32, name="gt")
                nc.scalar.activation(
                    out=gt[:], in_=z_t[:], func=Act.Tanh, bias=zero_bias[:, 0:1]
                )
                nc.vector.tensor_scalar(gt[:], gt[:], 1.0, None, op0=Alu.add)
                for k in range(K):
                    nc.vector.tensor_scalar_mul(
                        xts[m][:, k], in0=xts[m][:, k], scalar1=gt[:, k : k + 1]
                    )

            for n in range(NPC):
                xt = xts[n]
                u_t = small.tile([P, K], f32)   # raw per-channel sumsq
                # Stage-A engine split: ScalarE alone (2.8us/half) falls
                # ~1.9us/sample behind the DMA load cadence and ends up
                # gating the final stores.  Even samples run both halves on
                # ScalarE (ACT square+accum); odd samples run k=0 on ScalarE
                # and k=1 on the DVE (fused mult+row-sum).
                if n % 2 == 0:
                    ks = range(K)
                else:
                    ks = (0,)
                    nc.vector.scalar_tensor_tensor(
                        out=sq_dve[:],
                        in0=xt[:, 1],
                        scalar=1.0,
                        in1=xt[:, 1],
                        op0=Alu.mult,
                        op1=Alu.mult,
                        accum_out=u_t[:, 1:2],
                    )
                for k in ks:
                    nc.scalar.activation(
                        out=sq_act[:, k],
                        in_=xt[:, k],
                        func=Act.Square,
                        bias=zero_bias[:, 0:1],
                        accum_out=u_t[:, k : k + 1],
                    )
                if n < 2:
                    load(n + 6)
                if n >= 1:
                    finish(n - 1)
                    store(n - 1)

                # ---- stage B (tiny, per sample, fp32); u_t holds raw
                # sumsq, the +EPS folds into the two-scalar ops below ----
                ua = small.tile([P, K], f32)     # embed^2 = (u+eps) * alpha^2
                nc.vector.scalar_tensor_tensor(
                    out=ua[:], in0=u_t[:], scalar=EPS, in1=a2_col[:],
                    op0=Alu.add, op1=Alu.mult,
                )

                # col-sum of embed^2 broadcast to all partitions via PE
                cs = ps.tile([P, K], f32)
                nc.tensor.matmul(cs[:], ones_t[:], ua[:], start=True, stop=True)
                msum = small.tile([P, 1], f32)
                nc.vector.tensor_reduce(
                    msum[:], cs[:], axis=mybir.AxisListType.X, op=Alu.add
                )

                # v = mean + eps ; w = (u + eps) / v
                v_t = small.tile([P, 1], f32)
                nc.vector.tensor_scalar(
                    v_t[:], msum[:], 1.0 / C, EPS, op0=Alu.mult, op1=Alu.add
                )
                rv = small.tile([P, 1], f32)
                nc.vector.reciprocal(rv[:], v_t[:])
                w_t = small.tile([P, K], f32)
                nc.vector.tensor_scalar(
                    w_t[:], u_t[:], EPS, rv[:, 0:1], op0=Alu.add, op1=Alu.mult
                )

                # y ~= rsqrt(w): bit-trick seed + 1 Newton step (seed err
                # ~3% -> ~0.2%, giving |dz| <~ 1e-3 on z — far inside the
                # 2.4e-3 bf16 transport error that dominates the budget)
                y_t = small.tile([P, K], f32)
                sh = small.tile([P, K], u32)
                nc.vector.tensor_scalar(
                    sh[:], w_t[:].bitcast(u32), 1, None, op0=Alu.logical_shift_right
                )
                nc.vector.tensor_tensor(
                    out=y_t[:].bitcast(u32), in0=magic[:], in1=sh[:], op=Alu.subtract
                )
                t_t = small.tile([P, K], f32)
                for _ in range(1):
                    nc.vector.tensor_mul(t_t[:], w_t[:], y_t[:])
                    nc.vector.tensor_mul(t_t[:], t_t[:], y_t[:])
                    nc.vector.tensor_scalar(
                        t_t[:], t_t[:], -0.5, 1.5, op0=Alu.mult, op1=Alu.add
                    )
                    nc.vector.tensor_mul(y_t[:], y_t[:], t_t[:])

                # z = alpha*gamma*sqrt(w) + beta ;  sqrt(w) = w * rsqrt(w)
                z_t = small.tile([P, K], f32)
                nc.vector.tensor_mul(z_t[:], w_t[:], y_t[:])
                nc.vector.tensor_mul(z_t[:], z_t[:], ag_col[:])
                nc.vector.tensor_add(z_t[:], z_t[:], b_col[:])
                z_ts[n] = z_t

            finish(NPC - 1)
            store(NPC - 1)

    return _patch_bass(nc)


def _get_nc():
    if "nc" not in _cache:
        _cache["nc"] = _build()
    return _cache["nc"]


def _ensure_axon_hooks_stub():
    """bass_utils imports antenv.axon_hooks when tracing is requested (e.g.
    via a stray BASS_TRACE=1); this image lacks that module. Provide a stub
    whose hook getter returns None so the untraced fallback path runs."""
    import sys
    import types

    try:
        import antenv.axon_hooks  # noqa: F401
    except ImportError:
        mod = types.ModuleType("antenv.axon_hooks")
        _holder = [None]
        mod.set_axon_ntff_profile_hook = lambda h: _holder.__setitem__(0, h)
        mod.get_axon_ntff_profile_hook = lambda: _holder[0]
        sys.modules["antenv.axon_hooks"] = mod


def _run(x, alpha, gamma, beta, trace=False, **spmd_kwargs):
    import ml_dtypes

    from concourse.bass_utils import run_bass_kernel_spmd

    _ensure_axon_hooks_stub()

    nc = _get_nc()
    bf16 = ml_dtypes.bfloat16
    x = np.asarray(x).reshape(N, C, HW).astype(bf16)
    alpha = np.ascontiguousarray(np.asarray(alpha), dtype=np.float32)
    gamma = np.ascontiguousarray(np.asarray(gamma), dtype=np.float32)
    beta = np.ascontiguousarray(np.asarray(beta), dtype=np.float32)
    in_maps = [
        {
            "x": np.ascontiguousarray(x[c * NPC : (c + 1) * NPC]),
            "alpha": alpha,
            "gamma": gamma,
            "beta": beta,
        }
        for c in range(NCORES)
    ]
    res = run_bass_kernel_spmd(
        nc, in_maps, core_ids=list(range(NCORES)), trace=trace, **spmd_kwargs
    )
    full = np.concatenate([r["out"] for r in res.results], axis=0)
    return full.reshape(N, C, H, W).astype(np.float32), res


def kernel(x, alpha, gamma, beta):
    out, _ = _run(x, alpha, gamma, beta)
    return out
